# revision 2
# baseline (speedup 1.0000x reference)
"""Bahdanau additive attention on 8 TRN2 NeuronCores (batch-parallel).

Math: scores[b,i,j] = q[b,i].w + k[b,j].w, masked to -1e9 where mask==0,
softmax over j, then @ value.  The query term q[b,i].w is constant along j,
so it cancels in the softmax:

    out[b,i,:] = (sum_j mask[b,i,j] * e[b,j] * value[b,j,:])
               / (sum_j mask[b,i,j] * e[b,j]),      e[b,j] = exp(k[b,j].w)

(no query needed, no [Lq,Lk] softmax).  Per core: one batch.

Key idea vs the earlier u8->f16 converter design: the host uploads the
pre-transposed mask directly as fp8e4 bytes (0x00 / 0x38 = 0.0 / 1.0), so
the mask is a PE-ready stationary operand straight out of the DMA -- no
on-chip conversion at all (the old design burned DVE/ACT/SWDGE time and
supply latency converting 4M u8 elements per core, stalling the PE).
The e_j scale rides the moving operand instead: ev rows [e*v | e | 0]
(258 wide, even) built per strip by one DVE/ACT op from host-packed
[v | 1 | 0] records.  matmul(psum[i, 0:258] += maskT[j,i] * ev[j,:])
accumulates over 16 j-strips; col 256 is the softmax denominator.

Schedule: wave A = i-tiles 0-7 strip-major (strip s is needed ~s*0.9us
into the stream, matching DMA arrival order); wave B = i-tiles 8-15
tile-major (mask fully resident by then), so each tile's reciprocal +
scale + store streams out behind the PE instead of piling into a tail.
Two HWDGE rings in consumption order: sync carries kv chunks then
stores, scalar carries the 8 two-strip mask chunks then stores.  A
dependency-free burst of warm matmuls at kernel start trips the PE HAM
activity monitor to full clock before real work arrives.
"""

import os
import sys
import types

sys.path.insert(0, "/opt/trn_rl_repo")

import numpy as np

import concourse.bacc as bacc
import concourse.tile as tile
from concourse import mybir
from concourse.bass_utils import run_bass_kernel_spmd


def _ensure_ntff_hook_importable():
    """bass_utils imports antenv.axon_hooks when BASS_TRACE is set; this
    image's antenv lacks that module.  Provide it (and register the real
    ctypes NTFF hook if available) so tracing works instead of crashing."""
    if "antenv.axon_hooks" in sys.modules:
        return
    try:
        import antenv
    except ImportError:
        return
    hooks = types.ModuleType("antenv.axon_hooks")
    hooks._hook = None
    hooks.set_axon_ntff_profile_hook = lambda h: setattr(hooks, "_hook", h)
    hooks.get_axon_ntff_profile_hook = lambda: hooks._hook
    sys.modules["antenv.axon_hooks"] = hooks
    antenv.axon_hooks = hooks
    try:
        from trn_agent_boot.trn_boot import _ntff_profile_via_ctypes

        hook = _ntff_profile_via_ctypes("/opt/axon/libaxon_pjrt.so")
        if hook is not None:
            hooks.set_axon_ntff_profile_hook(hook)
    except Exception:
        pass


_ensure_ntff_hook_importable()

P = 128
B = 8
L = 2048
D = 256
NT = L // P  # 16 strips / i-tiles per dim
NE = D + 2  # 258 = value cols + e col + zero pad (even moving width)

# packed wrep/k/v record geometry, in fp16 elements per partition
KV_WREP = D  # wrep: 256 f16
KV_REC = D + NE  # per strip: k 256 f16 + [v | 1 | 0] 258 f16
KV_TOT = KV_WREP + NT * KV_REC

N_WARM_FREE = 10  # dep-free warm matmuls (N=512) at kernel start
N_WARM_KV = 6  # warm matmuls gated on the kv head DMA

MASK_STRIPS_PER_CHUNK = 2  # mask DMA granularity (2 strips = 0.52 MB)
N_MASK_CHUNKS = NT // MASK_STRIPS_PER_CHUNK

LAST_RESULTS = None


def _build_nc():
    dt = mybir.dt
    nc = bacc.Bacc("TRN2", target_bir_lowering=False, debug=False, num_devices=B)

    maskt_d = nc.dram_tensor("maskt", [P, NT * L], dt.float8e4, kind="ExternalInput").ap()
    kv_d = nc.dram_tensor("kv", [P, KV_TOT], dt.float16, kind="ExternalInput").ap()
    out_d = nc.dram_tensor("out", [P, NT * D], dt.float16, kind="ExternalOutput").ap()

    with tile.TileContext(nc) as tc:
        with (
            tc.tile_pool(name="const", bufs=1) as const_pool,
            tc.tile_pool(name="kv", bufs=1) as kv_pool,
            tc.tile_pool(name="small", bufs=1) as small_pool,
            tc.tile_pool(name="junk", bufs=2) as junk_pool,
            tc.tile_pool(name="outp", bufs=2) as out_pool,
            tc.tile_pool(name="rec", bufs=16) as rec_pool,
            tc.tile_pool(name="acc", bufs=8, space="PSUM") as acc_pool,
        ):
            # HAM warmup: dummy matmuls with no real dependencies (zeroed
            # data; results never read) to bring the PE to full clock.
            warm_mv = const_pool.tile([P, 512], dt.float16)
            nc.vector.memset(warm_mv[:], 0.0)
            warm_ps = acc_pool.tile([P, 512], dt.float32, tag="acc", name="warm")
            for _ in range(N_WARM_FREE):
                nc.tensor.matmul(
                    warm_ps[:], warm_mv[:, 0:P], warm_mv[:], start=True, stop=True
                )

            kv_sb = kv_pool.tile([P, KV_TOT], dt.float16, tag="kvsb")
            wrep = kv_sb[:, 0:KV_WREP]

            def k_ap(s):
                o = KV_WREP + s * KV_REC
                return kv_sb[:, o : o + D]

            def vx_ap(s):
                o = KV_WREP + s * KV_REC + D
                return kv_sb[:, o : o + NE]

            m8 = kv_pool.tile([P, NT * L], dt.float8e4, tag="m8")

            # sync ring: packed kv chunks in strip order (head = wrep+s0-3)
            kv_edges = [0] + [KV_WREP + 4 * (c + 1) * KV_REC for c in range(4)]
            for c in range(4):
                sl = slice(kv_edges[c], kv_edges[c + 1])
                nc.sync.dma_start(kv_sb[:, sl], kv_d[:, sl])

            # scalar ring: mask chunks in strip (= consumption) order
            for c in range(N_MASK_CHUNKS):
                sl = slice(
                    c * MASK_STRIPS_PER_CHUNK * L, (c + 1) * MASK_STRIPS_PER_CHUNK * L
                )
                nc.scalar.dma_start(m8[:, sl], maskt_d[:, sl])

            # second warm burst, gated on the kv head DMA via its operands:
            # bridges the PE-idle gap between the dep-free burst and the
            # first real matmul so the HAM activity window never rethrottles
            for _ in range(N_WARM_KV):
                nc.tensor.matmul(
                    warm_ps[:], kv_sb[:, 0:P], kv_sb[:, 0:512], start=True, stop=True
                )

            # ---- prologue per chunk of 4 strips: sk = k.w ; e = exp(sk) ;
            # ev rows [e*v | e | 0] from the host-packed [v | 1 | 0].
            sk = small_pool.tile([P, NT], dt.float32, tag="sk")
            e_sb = small_pool.tile([P, NT], dt.float32, tag="e")
            ev = kv_pool.tile([P, NT * NE], dt.float16, tag="ev")
            ev3 = ev[:].rearrange("p (s n) -> p s n", n=NE)

            for c in range(4):
                with tc.high_priority():
                    for s in range(4 * c, 4 * c + 4):
                        junk = junk_pool.tile([P, D], dt.float16, tag="junk")
                        nc.vector.scalar_tensor_tensor(
                            out=junk[:],
                            in0=k_ap(s),
                            scalar=1.0,
                            in1=wrep,
                            op0=mybir.AluOpType.mult,
                            op1=mybir.AluOpType.mult,
                            accum_out=sk[:, s : s + 1],
                        )
                        if c == 0:
                            # per-strip exp + scale so strip s is ready the
                            # moment its sk lands (ev0 gates the stream head)
                            nc.scalar.activation(
                                e_sb[:, s : s + 1],
                                sk[:, s : s + 1],
                                mybir.ActivationFunctionType.Exp,
                            )
                            nc.vector.tensor_scalar_mul(
                                ev3[:, s, 0:NE], vx_ap(s), e_sb[:, s : s + 1]
                            )
                    if c > 0:
                        cs = slice(4 * c, 4 * c + 4)
                        nc.scalar.activation(
                            e_sb[:, cs], sk[:, cs], mybir.ActivationFunctionType.Exp
                        )
                        for s in range(4 * c, 4 * c + 4):
                            if s % 2 == 0:
                                nc.vector.tensor_scalar_mul(
                                    ev3[:, s, 0:NE], vx_ap(s), e_sb[:, s : s + 1]
                                )
                            else:
                                nc.scalar.mul(
                                    ev3[:, s, 0:NE], vx_ap(s), e_sb[:, s : s + 1]
                                )

            def mask_tile(s, t):
                return m8[:, s * L + t * P : s * L + (t + 1) * P]

            def epilogue(acc, t, outb, ti):
                rec = rec_pool.tile([P, 1], dt.float32, tag="rec", name=f"r{t}")
                nc.vector.reciprocal(rec[:], acc[:, D : D + 1])
                if ti % 2 == 0:
                    nc.scalar.mul(outb[:, ti * D : (ti + 1) * D], acc[:, 0:D], rec[:])
                else:
                    nc.vector.tensor_scalar_mul(
                        outb[:, ti * D : (ti + 1) * D], acc[:, 0:D], rec[:]
                    )

            # ---- wave A: i-tiles 0-7, strip-major (matches DMA arrival)
            accs = [
                acc_pool.tile([P, NE], dt.float32, tag="acc", name=f"acc{t}")
                for t in range(8)
            ]
            outbA = out_pool.tile([P, 8 * D], dt.float16, tag="outb", name="outbA")
            for s in range(NT):
                mov = ev3[:, s, 0:NE]
                for t in range(8):
                    nc.tensor.matmul(
                        accs[t][:],
                        mask_tile(s, t),
                        mov,
                        start=(s == 0),
                        stop=(s == NT - 1),
                    )
            for t in range(8):
                epilogue(accs[t], t, outbA, t)
                if t == 3:
                    nc.sync.dma_start(out_d[:, 0 : 4 * D], outbA[:, 0 : 4 * D])
                elif t == 7:
                    nc.scalar.dma_start(
                        out_d[:, 4 * D : 8 * D], outbA[:, 4 * D : 8 * D]
                    )

            # ---- wave B: i-tiles 8-15, tile-major (mask fully resident);
            # each tile's epilogue + store streams behind the PE.
            outbB = out_pool.tile([P, 8 * D], dt.float16, tag="outb", name="outbB")
            for t in range(8, NT):
                ti = t - 8
                accB = acc_pool.tile([P, NE], dt.float32, tag="acc", name=f"acc{t}")
                for s in range(NT):
                    nc.tensor.matmul(
                        accB[:],
                        mask_tile(s, t),
                        ev3[:, s, 0:NE],
                        start=(s == 0),
                        stop=(s == NT - 1),
                    )
                epilogue(accB, t, outbB, ti)
                if ti % 2 == 1:
                    eng = nc.sync if (ti // 2) % 2 == 0 else nc.scalar
                    eng.dma_start(
                        out_d[:, (t - 1) * D : (t + 1) * D],
                        outbB[:, (ti - 1) * D : (ti + 1) * D],
                    )

    nc.compile()
    return nc


def kernel(query, key, value, mask, w_align):
    global LAST_RESULTS
    key = np.asarray(key, dtype=np.float32)
    value = np.asarray(value, dtype=np.float32)
    mask = np.asarray(mask)
    w_align = np.asarray(w_align, dtype=np.float32)

    import ml_dtypes

    nc = _build_nc()
    in_maps = []
    for b in range(B):
        # maskt[p, s*L + t*128+c] = mask[b][i=128t+c, j=128s+p], as fp8e4
        # bytes: 0x38 is 1.0 in fp8_e4m3 (bias 7)
        mt = (
            (mask[b].astype(np.uint8) * np.uint8(0x38))
            .reshape(NT, P, NT, P)  # [t, c, s, p]
            .transpose(3, 2, 0, 1)  # [p, s, t, c]
            .reshape(P, NT * L)
        )
        # packed wrep/k/v records, all fp16
        kvb = np.zeros((P, KV_TOT), dtype=np.float16)
        kvb[:, 0:KV_WREP] = w_align[None, :]
        kb = key[b].reshape(NT, P, D).transpose(1, 0, 2)  # [p, s, d]
        vb = value[b].reshape(NT, P, D).transpose(1, 0, 2)
        for s in range(NT):
            o = KV_WREP + s * KV_REC
            kvb[:, o : o + D] = kb[:, s]
            kvb[:, o + D : o + D + D] = vb[:, s]
            kvb[:, o + D + D] = 1.0  # ones col -> e in moving col 256
            # col 257 stays 0 (pad for even moving width)
        in_maps.append(
            {
                "maskt": np.ascontiguousarray(mt).view(ml_dtypes.float8_e4m3),
                "kv": kvb,
            }
        )
    try:
        res = run_bass_kernel_spmd(nc, in_maps, core_ids=list(range(B)))
    except Exception:
        # e.g. trace requested but profiling unavailable -- retry untraced
        os.environ["BASS_NEVER_TRACE"] = "1"
        res = run_bass_kernel_spmd(nc, in_maps, core_ids=list(range(B)))
    LAST_RESULTS = res
    out = np.empty((B, L, D), dtype=np.float32)
    for b in range(B):
        ob = res.results[b]["out"].astype(np.float32)  # [p, t*D]
        out[b] = ob.reshape(P, NT, D).transpose(1, 0, 2).reshape(L, D)
    return out


# revision 5
# speedup vs baseline: 1.2084x; 1.2084x over previous
"""Bahdanau additive attention on 8 TRN2 NeuronCores (batch-parallel).

Math: scores[b,i,j] = q[b,i].w + k[b,j].w, masked to -1e9 where mask==0,
softmax over j, then @ value.  The query term q[b,i].w is constant along j,
so it cancels in the softmax:

    out[b,i,:] = (sum_j mask[b,i,j] * e[b,j] * value[b,j,:])
               / (sum_j mask[b,i,j] * e[b,j]),      e[b,j] = exp(k[b,j].w)

(no query needed, no [Lq,Lk] softmax).  Per core: one batch.

The host uploads the pre-transposed mask directly as fp8e4 bytes
(0x00 / 0x38 = 0.0 / 1.0), so the mask is a PE-ready stationary operand
straight out of the DMA -- no on-chip conversion at all, and (measured)
fp8 LDWEIGHTS is ~97ns, under the 258-col matmul's 108ns, so the stream
runs at the fp16 rate of ~112ns/MM.  k and v stay fp16: any quantization
of k or v is multiplicative noise on the softmax weights / values, and
the output (a random walk in v) inherits it 1:1 -- fp8 k measured 3e-2
rel err.  The e_j scale rides the moving operand: ev rows [e*v | e | 0]
(258 wide) built per strip by one DVE/ACT op from host-packed [v | 1 | 0]
records.  matmul(psum[i, 0:258] += maskT[j,i] * ev[j,:]) accumulates
over 16 j-strips; col 256 is the softmax denominator.

Supply is the whole game: wave A (i-tiles 0-7, strip-major) consumes
mask+k+v bytes at ~430 GB/s -- over the per-core HBM limit -- so wave A
is DMA-paced and ALL loads go on ONE HWDGE ring (sync) in exact
consumption order; no compute op ever queues ahead of a load dispatch
on that engine.  A tiny head tensor [w | k0 | vx0] starts the stream
~1us earlier than a monolithic kv block.  Zero-matmuls (+0 into an
accumulator, exact no-op) after each wave-A strip absorb the DMA-paced
idle so the HAM activity window never rethrottles the PE clock to 1.2
GHz mid-kernel.  Wave B (i-tiles 8-15) is tile-major: mask is fully
resident by then, and each tile's reciprocal + scale + store streams
out behind the PE instead of piling into a tail.
"""

import os
import sys
import types

sys.path.insert(0, "/opt/trn_rl_repo")

import numpy as np

import concourse.bacc as bacc
import concourse.tile as tile
from concourse import mybir
from concourse.bass_utils import run_bass_kernel_spmd


def _ensure_ntff_hook_importable():
    """bass_utils imports antenv.axon_hooks when BASS_TRACE is set; this
    image's antenv lacks that module.  Provide it (and register the real
    ctypes NTFF hook if available) so tracing works instead of crashing."""
    if "antenv.axon_hooks" in sys.modules:
        return
    try:
        import antenv
    except ImportError:
        return
    hooks = types.ModuleType("antenv.axon_hooks")
    hooks._hook = None
    hooks.set_axon_ntff_profile_hook = lambda h: setattr(hooks, "_hook", h)
    hooks.get_axon_ntff_profile_hook = lambda: hooks._hook
    sys.modules["antenv.axon_hooks"] = hooks
    antenv.axon_hooks = hooks
    try:
        from trn_agent_boot.trn_boot import _ntff_profile_via_ctypes

        hook = _ntff_profile_via_ctypes("/opt/axon/libaxon_pjrt.so")
        if hook is not None:
            hooks.set_axon_ntff_profile_hook(hook)
    except Exception:
        pass


_ensure_ntff_hook_importable()

P = 128
B = 8
L = 2048
D = 256
NT = L // P  # 16 strips / i-tiles per dim
NE = D + 2  # 258 = value cols + e col + zero pad (even moving width)

HEAD_TOT = D + D + NE  # w | k0 | vx0
KVR_REC = D + NE  # per strip s>=1: k_s | [v|1|0]_s
KVR_TOT = (NT - 1) * KVR_REC

# strip groups: ev_s for a group becomes computable when its kv chunk lands
KV_GROUPS = ((0,), (1, 2, 3), (4, 5, 6, 7), (8, 9, 10, 11), (12, 13, 14, 15))

N_WARM_FREE = 10  # dep-free warm matmuls (N=512) at kernel start
N_WARM_KV = 5  # warm matmuls gated on the head DMA

LAST_RESULTS = None


def _build_nc():
    dt = mybir.dt
    nc = bacc.Bacc("TRN2", target_bir_lowering=False, debug=False, num_devices=B)

    maskt_d = nc.dram_tensor("maskt", [P, NT * L], dt.float8e4, kind="ExternalInput").ap()
    head_d = nc.dram_tensor("head", [P, HEAD_TOT], dt.float16, kind="ExternalInput").ap()
    kvr_d = nc.dram_tensor("kvr", [P, KVR_TOT], dt.float16, kind="ExternalInput").ap()
    out_d = nc.dram_tensor("out", [P, NT * D], dt.float16, kind="ExternalOutput").ap()

    with tile.TileContext(nc) as tc:
        with (
            tc.tile_pool(name="const", bufs=1) as const_pool,
            tc.tile_pool(name="kv", bufs=1) as kv_pool,
            tc.tile_pool(name="small", bufs=1) as small_pool,
            tc.tile_pool(name="junk", bufs=2) as junk_pool,
            tc.tile_pool(name="outp", bufs=2) as out_pool,
            tc.tile_pool(name="rec", bufs=16) as rec_pool,
            tc.tile_pool(name="acc", bufs=8, space="PSUM") as acc_pool,
        ):
            # HAM warmup: dummy matmuls with no real dependencies to bring
            # the PE to full clock before data arrives.
            warm_mv = const_pool.tile([P, 512], dt.float16)
            nc.vector.memset(warm_mv[:], 0.0)
            warm_ps = acc_pool.tile([P, 512], dt.float32, tag="acc", name="warm")
            for _ in range(N_WARM_FREE):
                nc.tensor.matmul(
                    warm_ps[:], warm_mv[:, 0:P], warm_mv[:], start=True, stop=True
                )

            m8 = kv_pool.tile([P, NT * L], dt.float8e4, tag="m8")
            head = kv_pool.tile([P, HEAD_TOT], dt.float16, tag="head")
            kvr = kv_pool.tile([P, KVR_TOT], dt.float16, tag="kvr")
            wrep = head[:, 0:D]

            def k_ap(s):
                if s == 0:
                    return head[:, D : 2 * D]
                o = (s - 1) * KVR_REC
                return kvr[:, o : o + D]

            def vx_ap(s):
                if s == 0:
                    return head[:, 2 * D : 2 * D + NE]
                o = (s - 1) * KVR_REC + D
                return kvr[:, o : o + NE]

            # ---- THE load ring (sync), in exact consumption order.
            def m_load(s_lo, s_hi):
                sl = slice(s_lo * L, (s_hi + 1) * L)
                nc.sync.dma_start(m8[:, sl], maskt_d[:, sl])

            def kv_load(gi):
                g = KV_GROUPS[gi]
                sl = slice((g[0] - 1) * KVR_REC, g[-1] * KVR_REC)
                nc.sync.dma_start(kvr[:, sl], kvr_d[:, sl])

            nc.sync.dma_start(head[:], head_d[:])  # w | k0 | vx0, 0.20 MB
            m_load(0, 0)
            kv_load(1)  # s1-3
            m_load(1, 2)
            m_load(3, 4)
            kv_load(2)  # s4-7
            m_load(5, 6)
            m_load(7, 8)
            kv_load(3)  # s8-11
            m_load(9, 10)
            m_load(11, 12)
            kv_load(4)  # s12-15
            m_load(13, 14)
            m_load(15, 15)

            # second warm burst, gated on the head DMA via its operands:
            # bridges the PE-idle gap to the first real matmul.
            for _ in range(N_WARM_KV):
                nc.tensor.matmul(
                    warm_ps[:], head[:, 0:P], head[:, 0:512], start=True, stop=True
                )

            # ---- prologue per kv group: sk = k.w ; e = exp(sk) ;
            # ev rows [e*v | e | 0] from the host-packed [v | 1 | 0].
            sk = small_pool.tile([P, NT], dt.float32, tag="sk")
            e_sb = small_pool.tile([P, NT], dt.float32, tag="e")
            ev = kv_pool.tile([P, NT * NE], dt.float16, tag="ev")
            ev3 = ev[:].rearrange("p (s n) -> p s n", n=NE)

            for gi, g in enumerate(KV_GROUPS):
                with tc.high_priority():
                    for s in g:
                        junk = junk_pool.tile([P, D], dt.float16, tag="junk")
                        nc.vector.scalar_tensor_tensor(
                            out=junk[:],
                            in0=k_ap(s),
                            scalar=1.0,
                            in1=wrep,
                            op0=mybir.AluOpType.mult,
                            op1=mybir.AluOpType.mult,
                            accum_out=sk[:, s : s + 1],
                        )
                        if gi <= 1:
                            # per-strip exp + scale: strip s ready the moment
                            # its sk lands (these gate the stream head)
                            nc.scalar.activation(
                                e_sb[:, s : s + 1],
                                sk[:, s : s + 1],
                                mybir.ActivationFunctionType.Exp,
                            )
                            if s % 2 == 0:
                                nc.vector.tensor_scalar_mul(
                                    ev3[:, s, 0:NE], vx_ap(s), e_sb[:, s : s + 1]
                                )
                            else:
                                nc.scalar.mul(
                                    ev3[:, s, 0:NE], vx_ap(s), e_sb[:, s : s + 1]
                                )
                    if gi > 1:
                        cs = slice(g[0], g[-1] + 1)
                        nc.scalar.activation(
                            e_sb[:, cs], sk[:, cs], mybir.ActivationFunctionType.Exp
                        )
                        for s in g:
                            if s % 2 == 0:
                                nc.vector.tensor_scalar_mul(
                                    ev3[:, s, 0:NE], vx_ap(s), e_sb[:, s : s + 1]
                                )
                            else:
                                nc.scalar.mul(
                                    ev3[:, s, 0:NE], vx_ap(s), e_sb[:, s : s + 1]
                                )

            def mask_tile(s, t):
                return m8[:, s * L + t * P : s * L + (t + 1) * P]

            def epilogue(acc, t, outb, ti, split=False):
                rec = rec_pool.tile([P, 1], dt.float32, tag="rec", name=f"r{t}")
                nc.vector.reciprocal(rec[:], acc[:, D : D + 1])
                ob = outb[:, ti * D : (ti + 1) * D]
                if split:
                    # last tile: split the scale across DVE+ACT to shorten
                    # the post-stream tail
                    nc.vector.tensor_scalar_mul(ob[:, 0:128], acc[:, 0:128], rec[:])
                    nc.scalar.mul(ob[:, 128:D], acc[:, 128:D], rec[:])
                elif ti % 2 == 0:
                    nc.scalar.mul(ob, acc[:, 0:D], rec[:])
                else:
                    nc.vector.tensor_scalar_mul(ob, acc[:, 0:D], rec[:])

            # ---- wave A: i-tiles 0-7, strip-major (matches DMA arrival).
            # Wave A is DMA-paced; zero-matmuls (+0 accumulate, exact no-op)
            # after each strip absorb the idle and keep the HAM window busy.
            accs = [
                acc_pool.tile([P, NE], dt.float32, tag="acc", name=f"acc{t}")
                for t in range(8)
            ]
            outbA = out_pool.tile([P, 8 * D], dt.float16, tag="outb", name="outbA")
            for s in range(NT):
                mov = ev3[:, s, 0:NE]
                for t in range(8):
                    nc.tensor.matmul(
                        accs[t][:],
                        mask_tile(s, t),
                        mov,
                        start=(s == 0),
                        stop=(s == NT - 1),
                    )
                if 1 <= s <= 13:
                    for _ in range(2):
                        nc.tensor.matmul(
                            accs[7][:],
                            warm_mv[:, 0:P],
                            warm_mv[:, 0:NE],
                            start=False,
                            stop=False,
                        )
            for t in range(8):
                epilogue(accs[t], t, outbA, t)
                if t == 3:
                    nc.sync.dma_start(out_d[:, 0 : 4 * D], outbA[:, 0 : 4 * D])
                elif t == 7:
                    nc.sync.dma_start(out_d[:, 4 * D : 8 * D], outbA[:, 4 * D : 8 * D])

            # ---- wave B: i-tiles 8-15, tile-major (mask fully resident);
            # each tile's epilogue + store streams behind the PE.
            outbB = out_pool.tile([P, 8 * D], dt.float16, tag="outb", name="outbB")
            for t in range(8, NT):
                ti = t - 8
                accB = acc_pool.tile([P, NE], dt.float32, tag="acc", name=f"acc{t}")
                for s in range(NT):
                    nc.tensor.matmul(
                        accB[:],
                        mask_tile(s, t),
                        ev3[:, s, 0:NE],
                        start=(s == 0),
                        stop=(s == NT - 1),
                    )
                epilogue(accB, t, outbB, ti, split=(t == NT - 1))
                # stores: pairs for tiles 8-13, singles for 14/15 so the
                # final store (and its HBM write receipt) is small and early
                if t in (9, 11, 13):
                    nc.sync.dma_start(
                        out_d[:, (t - 1) * D : (t + 1) * D],
                        outbB[:, (ti - 1) * D : (ti + 1) * D],
                    )
                elif t >= 14:
                    nc.sync.dma_start(
                        out_d[:, t * D : (t + 1) * D],
                        outbB[:, ti * D : (ti + 1) * D],
                    )

    nc.compile()
    return nc


def kernel(query, key, value, mask, w_align):
    global LAST_RESULTS
    key = np.asarray(key, dtype=np.float32)
    value = np.asarray(value, dtype=np.float32)
    mask = np.asarray(mask)
    w_align = np.asarray(w_align, dtype=np.float32)

    import ml_dtypes

    nc = _build_nc()
    in_maps = []
    for b in range(B):
        # maskt[p, s*L + t*128+c] = mask[b][i=128t+c, j=128s+p], as fp8e4
        # bytes: 0x38 is 1.0 in fp8_e4m3 (bias 7)
        mt = (
            (mask[b].astype(np.uint8) * np.uint8(0x38))
            .reshape(NT, P, NT, P)  # [t, c, s, p]
            .transpose(3, 2, 0, 1)  # [p, s, t, c]
            .reshape(P, NT * L)
        )
        kb = key[b].reshape(NT, P, D).transpose(1, 0, 2)  # [p, s, d]
        vb = value[b].reshape(NT, P, D).transpose(1, 0, 2)
        headb = np.zeros((P, HEAD_TOT), dtype=np.float16)
        headb[:, 0:D] = w_align[None, :]
        headb[:, D : 2 * D] = kb[:, 0]
        headb[:, 2 * D : 2 * D + D] = vb[:, 0]
        headb[:, 2 * D + D] = 1.0
        kvrb = np.zeros((P, KVR_TOT), dtype=np.float16)
        for s in range(1, NT):
            o = (s - 1) * KVR_REC
            kvrb[:, o : o + D] = kb[:, s]
            kvrb[:, o + D : o + D + D] = vb[:, s]
            kvrb[:, o + D + D] = 1.0  # ones col -> e in moving col 256
            # col 257 stays 0 (pad for even moving width)
        in_maps.append(
            {
                "maskt": np.ascontiguousarray(mt).view(ml_dtypes.float8_e4m3),
                "head": headb,
                "kvr": kvrb,
            }
        )
    try:
        res = run_bass_kernel_spmd(nc, in_maps, core_ids=list(range(B)))
    except Exception:
        # e.g. trace requested but profiling unavailable -- retry untraced
        os.environ["BASS_NEVER_TRACE"] = "1"
        res = run_bass_kernel_spmd(nc, in_maps, core_ids=list(range(B)))
    LAST_RESULTS = res
    out = np.empty((B, L, D), dtype=np.float32)
    for b in range(B):
        ob = res.results[b]["out"].astype(np.float32)  # [p, t*D]
        out[b] = ob.reshape(P, NT, D).transpose(1, 0, 2).reshape(L, D)
    return out


# revision 16
# speedup vs baseline: 1.2230x; 1.0121x over previous
"""Bahdanau additive attention on 8 TRN2 NeuronCores (batch-parallel).

Math: scores[b,i,j] = q[b,i].w + k[b,j].w, masked to -1e9 where mask==0,
softmax over j, then @ value.  The query term q[b,i].w is constant along j,
so it cancels in the softmax:

    out[b,i,:] = (sum_j mask[b,i,j] * e[b,j] * value[b,j,:])
               / (sum_j mask[b,i,j] * e[b,j]),      e[b,j] = exp(k[b,j].w)

(no query needed, no [Lq,Lk] softmax).  Per core: one batch.

The host uploads the pre-transposed mask directly as fp8e4 bytes
(0x00 / 0x38 = 0.0 / 1.0), so the mask is a PE-ready stationary operand
straight out of the DMA -- no on-chip conversion at all, and (measured)
fp8 LDWEIGHTS is ~97ns, under the 258-col matmul's 108ns, so the stream
runs at the fp16 rate of ~112ns/MM.  k and v stay fp16: any quantization
of k or v is multiplicative noise on the softmax weights / values, and
the output (a random walk in v) inherits it 1:1 -- fp8 k measured 3e-2
rel err.  The e_j scale rides the moving operand: ev rows [e*v | e | 0]
(258 wide) built per strip by one DVE/ACT op from host-packed [v | 1 | 0]
records.  matmul(psum[i, 0:258] += maskT[j,i] * ev[j,:]) accumulates
over 16 j-strips; col 256 is the softmax denominator.

Supply is the whole game: wave A (i-tiles 0-7, strip-major) consumes
mask+k+v bytes at ~430 GB/s -- over the per-core HBM limit -- so wave A
is DMA-paced and ALL loads go on ONE HWDGE ring (sync) in exact
consumption order; no compute op ever queues ahead of a load dispatch
on that engine.  A tiny head tensor [w | k0 | vx0] starts the stream
~1us earlier than a monolithic kv block.  Zero-matmuls (+0 into an
accumulator, exact no-op) after each wave-A strip absorb the DMA-paced
idle so the HAM activity window never rethrottles the PE clock to 1.2
GHz mid-kernel.  Wave B (i-tiles 8-15) is tile-major: mask is fully
resident by then, and each tile's reciprocal + scale + store streams
out behind the PE instead of piling into a tail.
"""

import os
import sys
import types

sys.path.insert(0, "/opt/trn_rl_repo")

import numpy as np

import concourse.bacc as bacc
import concourse.tile as tile
from concourse import mybir
from concourse.bass_utils import run_bass_kernel_spmd


def _ensure_ntff_hook_importable():
    """bass_utils imports antenv.axon_hooks when BASS_TRACE is set; this
    image's antenv lacks that module.  Provide it (and register the real
    ctypes NTFF hook if available) so tracing works instead of crashing."""
    if "antenv.axon_hooks" in sys.modules:
        return
    try:
        import antenv
    except ImportError:
        return
    hooks = types.ModuleType("antenv.axon_hooks")
    hooks._hook = None
    hooks.set_axon_ntff_profile_hook = lambda h: setattr(hooks, "_hook", h)
    hooks.get_axon_ntff_profile_hook = lambda: hooks._hook
    sys.modules["antenv.axon_hooks"] = hooks
    antenv.axon_hooks = hooks
    try:
        from trn_agent_boot.trn_boot import _ntff_profile_via_ctypes

        hook = _ntff_profile_via_ctypes("/opt/axon/libaxon_pjrt.so")
        if hook is not None:
            hooks.set_axon_ntff_profile_hook(hook)
    except Exception:
        pass


_ensure_ntff_hook_importable()

P = 128
B = 8
L = 2048
D = 256
NT = L // P  # 16 strips / i-tiles per dim
NE = D + 2  # 258 = value cols + e col + zero pad (even moving width)

HEAD_TOT = D + D + NE  # w | k0 | vx0
KVR_REC = D + NE  # per strip s>=1: k_s | [v|1|0]_s
KVR_TOT = (NT - 1) * KVR_REC

# strip groups: ev_s for a group becomes computable when its kv chunk lands
KV_GROUPS = ((0,), (1, 2, 3), (4, 5, 6, 7), (8, 9, 10, 11), (12, 13, 14, 15))

N_UNPACK = 3  # strips NT-N_UNPACK..NT-1 arrive bit-packed, unpacked on DVE
UNPACK_S0 = NT - N_UNPACK

N_WARM_FREE = 7  # dep-free warm matmuls (N=512) at kernel start
N_WARM_KV = 1  # warm matmuls gated on the head DMA

LAST_RESULTS = None


def _build_nc():
    dt = mybir.dt
    nc = bacc.Bacc("TRN2", target_bir_lowering=False, debug=False, num_devices=B)

    maskt_d = nc.dram_tensor(
        "maskt", [P, UNPACK_S0 * L], dt.float8e4, kind="ExternalInput"
    ).ap()
    bits_d = nc.dram_tensor(
        "bits", [P, N_UNPACK * 256], dt.uint8, kind="ExternalInput"
    ).ap()
    head_d = nc.dram_tensor("head", [P, HEAD_TOT], dt.float16, kind="ExternalInput").ap()
    kvr_d = nc.dram_tensor("kvr", [P, KVR_TOT], dt.float16, kind="ExternalInput").ap()
    out_d = nc.dram_tensor("out", [P, NT * D], dt.float16, kind="ExternalOutput").ap()

    with tile.TileContext(nc) as tc:
        with (
            tc.tile_pool(name="const", bufs=1) as const_pool,
            tc.tile_pool(name="kv", bufs=1) as kv_pool,
            tc.tile_pool(name="small", bufs=1) as small_pool,
            tc.tile_pool(name="junk", bufs=2) as junk_pool,
            tc.tile_pool(name="outp", bufs=2) as out_pool,
            tc.tile_pool(name="rec", bufs=16) as rec_pool,
            tc.tile_pool(name="acc", bufs=8, space="PSUM") as acc_pool,
        ):
            # HAM warmup: dummy matmuls with no real dependencies to bring
            # the PE to full clock before data arrives.
            warm_mv = const_pool.tile([P, 512], dt.float16)
            nc.vector.memset(warm_mv[:], 0.0)
            warm_ps = acc_pool.tile([P, 512], dt.float32, tag="acc", name="warm")
            for _ in range(N_WARM_FREE):
                nc.tensor.matmul(
                    warm_ps[:], warm_mv[:, 0:P], warm_mv[:], start=True, stop=True
                )

            m8 = kv_pool.tile([P, NT * L], dt.float8e4, tag="m8")
            bits = kv_pool.tile([P, N_UNPACK * 256], dt.uint8, tag="bits")
            head = kv_pool.tile([P, HEAD_TOT], dt.float16, tag="head")
            kvr = kv_pool.tile([P, KVR_TOT], dt.float16, tag="kvr")
            wrep = head[:, 0:D]

            def k_ap(s):
                if s == 0:
                    return head[:, D : 2 * D]
                o = (s - 1) * KVR_REC
                return kvr[:, o : o + D]

            def vx_ap(s):
                if s == 0:
                    return head[:, 2 * D : 2 * D + NE]
                o = (s - 1) * KVR_REC + D
                return kvr[:, o : o + NE]

            # ---- THE load ring (sync), in exact consumption order.
            def m_load(s_lo, s_hi):
                sl = slice(s_lo * L, (s_hi + 1) * L)
                nc.sync.dma_start(m8[:, sl], maskt_d[:, sl])

            def kv_load(gi):
                g = KV_GROUPS[gi]
                sl = slice((g[0] - 1) * KVR_REC, g[-1] * KVR_REC)
                nc.sync.dma_start(kvr[:, sl], kvr_d[:, sl])

            nc.sync.dma_start(head[:], head_d[:])  # w | k0 | vx0, 0.20 MB
            nc.sync.dma_start(bits[:], bits_d[:])  # packed strips 13-15
            m_load(0, 0)
            kv_load(1)  # s1-3
            m_load(1, 2)
            kv_load(2)  # s4-7
            m_load(3, 4)
            m_load(5, 6)
            kv_load(3)  # s8-11
            m_load(7, 8)
            m_load(9, 10)
            kv_load(4)  # s12-15
            m_load(11, 12)

            # second warm burst, gated on the head DMA via its operands:
            # bridges the PE-idle gap to the first real matmul.
            for _ in range(N_WARM_KV):
                nc.tensor.matmul(
                    warm_ps[:], head[:, 0:P], head[:, 0:512], start=True, stop=True
                )

            # ---- prologue per kv group: sk = k.w ; e = exp(sk) ;
            # ev rows [e*v | e | 0] from the host-packed [v | 1 | 0].
            sk = small_pool.tile([P, NT], dt.float32, tag="sk")
            e_sb = small_pool.tile([P, NT], dt.float32, tag="e")
            ev = kv_pool.tile([P, NT * NE], dt.float16, tag="ev")
            ev3 = ev[:].rearrange("p (s n) -> p s n", n=NE)

            for gi, g in enumerate(KV_GROUPS):
                with tc.high_priority():
                    for s in g:
                        junk = junk_pool.tile([P, D], dt.float16, tag="junk")
                        nc.vector.scalar_tensor_tensor(
                            out=junk[:],
                            in0=k_ap(s),
                            scalar=1.0,
                            in1=wrep,
                            op0=mybir.AluOpType.mult,
                            op1=mybir.AluOpType.mult,
                            accum_out=sk[:, s : s + 1],
                        )
                        if gi <= 1:
                            # per-strip exp + scale: strip s ready the moment
                            # its sk lands (these gate the stream head)
                            nc.scalar.activation(
                                e_sb[:, s : s + 1],
                                sk[:, s : s + 1],
                                mybir.ActivationFunctionType.Exp,
                            )
                            if s % 2 == 0:
                                nc.vector.tensor_scalar_mul(
                                    ev3[:, s, 0:NE], vx_ap(s), e_sb[:, s : s + 1]
                                )
                            else:
                                nc.scalar.mul(
                                    ev3[:, s, 0:NE], vx_ap(s), e_sb[:, s : s + 1]
                                )
                    if gi > 1:
                        cs = slice(g[0], g[-1] + 1)
                        nc.scalar.activation(
                            e_sb[:, cs], sk[:, cs], mybir.ActivationFunctionType.Exp
                        )
                        for s in g:
                            if s % 2 == 0:
                                nc.vector.tensor_scalar_mul(
                                    ev3[:, s, 0:NE], vx_ap(s), e_sb[:, s : s + 1]
                                )
                            else:
                                nc.scalar.mul(
                                    ev3[:, s, 0:NE], vx_ap(s), e_sb[:, s : s + 1]
                                )

            # ---- unpack bit-packed strips 13-15 on DVE: plane b of strip s
            # covers i in [256b, 256b+256).  Two passes (the ISA rejects
            # mixed bitwise+arith in one op): bitwise and-mask to {0, 2^b}
            # u8, then one arith is_gt per strip emitting numeric {0.0, 1.0}
            # which the DVE output stage encodes as fp8e4 bytes 0x00/0x38.
            ub = kv_pool.tile([P, N_UNPACK * L], dt.uint8, tag="ub")
            for s in range(UNPACK_S0, NT):
                si = s - UNPACK_S0
                src = bits[:, si * 256 : (si + 1) * 256]
                for bpl in range(8):
                    nc.vector.tensor_scalar(
                        out=ub[:, si * L + 256 * bpl : si * L + 256 * (bpl + 1)],
                        in0=src,
                        scalar1=1 << bpl,
                        scalar2=None,
                        op0=mybir.AluOpType.bitwise_and,
                    )
                nc.vector.tensor_scalar(
                    out=m8[:, s * L : (s + 1) * L],
                    in0=ub[:, si * L : (si + 1) * L],
                    scalar1=0,
                    scalar2=None,
                    op0=mybir.AluOpType.is_gt,
                )

            def mask_tile(s, t):
                return m8[:, s * L + t * P : s * L + (t + 1) * P]

            def epilogue(acc, t, outb, ti):
                rec = rec_pool.tile([P, 1], dt.float32, tag="rec", name=f"r{t}")
                nc.vector.reciprocal(rec[:], acc[:, D : D + 1])
                ob = outb[:, ti * D : (ti + 1) * D]
                if ti % 2 == 0:
                    nc.scalar.mul(ob, acc[:, 0:D], rec[:])
                else:
                    nc.vector.tensor_scalar_mul(ob, acc[:, 0:D], rec[:])

            # ---- wave A: i-tiles 0-7, strip-major (matches DMA arrival).
            # Wave A is DMA-paced; zero-matmuls (+0 accumulate, exact no-op)
            # after each strip absorb the idle and keep the HAM window busy.
            accs = [
                acc_pool.tile([P, NE], dt.float32, tag="acc", name=f"acc{t}")
                for t in range(8)
            ]
            outbA = out_pool.tile([P, 8 * D], dt.float16, tag="outb", name="outbA")
            for s in range(NT):
                mov = ev3[:, s, 0:NE]
                for t in range(8):
                    nc.tensor.matmul(
                        accs[t][:],
                        mask_tile(s, t),
                        mov,
                        start=(s == 0),
                        stop=(s == NT - 1),
                    )
                if 1 <= s <= 12:
                    for _ in range(1):
                        nc.tensor.matmul(
                            accs[7][:],
                            warm_mv[:, 0:P],
                            warm_mv[:, 0:NE],
                            start=False,
                            stop=False,
                        )
            for t in range(8):
                epilogue(accs[t], t, outbA, t)
                if t == 3:
                    nc.sync.dma_start(out_d[:, 0 : 4 * D], outbA[:, 0 : 4 * D])
                elif t == 7:
                    nc.sync.dma_start(out_d[:, 4 * D : 8 * D], outbA[:, 4 * D : 8 * D])

            # ---- wave B: i-tiles 8-15, tile-major (mask fully resident);
            # each tile's epilogue + store streams behind the PE.
            outbB = out_pool.tile([P, 8 * D], dt.float16, tag="outb", name="outbB")
            for t in range(8, NT):
                ti = t - 8
                accB = acc_pool.tile([P, NE], dt.float32, tag="acc", name=f"acc{t}")
                for s in range(NT):
                    nc.tensor.matmul(
                        accB[:],
                        mask_tile(s, t),
                        ev3[:, s, 0:NE],
                        start=(s == 0),
                        stop=(s == NT - 1),
                    )
                epilogue(accB, t, outbB, ti)
                # stores: pairs for tiles 8-13, singles for 14/15 so the
                # final store (and its HBM write receipt) is small and early
                if t in (9, 11, 13):
                    nc.sync.dma_start(
                        out_d[:, (t - 1) * D : (t + 1) * D],
                        outbB[:, (ti - 1) * D : (ti + 1) * D],
                    )
                elif t >= 14:
                    nc.sync.dma_start(
                        out_d[:, t * D : (t + 1) * D],
                        outbB[:, ti * D : (ti + 1) * D],
                    )

    nc.compile()
    return nc


def kernel(query, key, value, mask, w_align):
    global LAST_RESULTS
    key = np.asarray(key, dtype=np.float32)
    value = np.asarray(value, dtype=np.float32)
    mask = np.asarray(mask)
    w_align = np.asarray(w_align, dtype=np.float32)

    import ml_dtypes

    nc = _build_nc()
    in_maps = []
    for b in range(B):
        # maskt[p, s*L + t*128+c] = mask[b][i=128t+c, j=128s+p], as fp8e4
        # bytes: 0x38 is 1.0 in fp8_e4m3 (bias 7)
        mu = mask[b].astype(np.uint8)  # [i, j]
        mt = (
            (mu * np.uint8(0x38))
            .reshape(NT, P, NT, P)  # [t, c, s, p]
            .transpose(3, 2, 0, 1)  # [p, s, t, c]
            .reshape(P, NT * L)[:, : UNPACK_S0 * L]
        )
        # bit-packed strips 13-15: bit b of bits[p, (s-13)*256 + w] is
        # mask[i = 256b + w, j = s*128 + p]
        bitsb = np.zeros((P, N_UNPACK * 256), dtype=np.uint8)
        sh = np.arange(8, dtype=np.uint8)[:, None, None]
        for s in range(UNPACK_S0, NT):
            blk = mu[:, s * P : (s + 1) * P]  # [2048 i, 128 p]
            packed = (blk.reshape(8, 256, P) << sh).sum(0).astype(np.uint8)
            bitsb[:, (s - UNPACK_S0) * 256 : (s - UNPACK_S0 + 1) * 256] = packed.T
        kb = key[b].reshape(NT, P, D).transpose(1, 0, 2)  # [p, s, d]
        vb = value[b].reshape(NT, P, D).transpose(1, 0, 2)
        headb = np.zeros((P, HEAD_TOT), dtype=np.float16)
        headb[:, 0:D] = w_align[None, :]
        headb[:, D : 2 * D] = kb[:, 0]
        headb[:, 2 * D : 2 * D + D] = vb[:, 0]
        headb[:, 2 * D + D] = 1.0
        kvrb = np.zeros((P, KVR_TOT), dtype=np.float16)
        for s in range(1, NT):
            o = (s - 1) * KVR_REC
            kvrb[:, o : o + D] = kb[:, s]
            kvrb[:, o + D : o + D + D] = vb[:, s]
            kvrb[:, o + D + D] = 1.0  # ones col -> e in moving col 256
            # col 257 stays 0 (pad for even moving width)
        in_maps.append(
            {
                "maskt": np.ascontiguousarray(mt).view(ml_dtypes.float8_e4m3),
                "bits": bitsb,
                "head": headb,
                "kvr": kvrb,
            }
        )
    try:
        res = run_bass_kernel_spmd(nc, in_maps, core_ids=list(range(B)))
    except Exception:
        # e.g. trace requested but profiling unavailable -- retry untraced
        os.environ["BASS_NEVER_TRACE"] = "1"
        res = run_bass_kernel_spmd(nc, in_maps, core_ids=list(range(B)))
    LAST_RESULTS = res
    out = np.empty((B, L, D), dtype=np.float32)
    for b in range(B):
        ob = res.results[b]["out"].astype(np.float32)  # [p, t*D]
        out[b] = ob.reshape(P, NT, D).transpose(1, 0, 2).reshape(L, D)
    return out
